# revision 15
# baseline (speedup 1.0000x reference)
# Trainium2 Bass kernel for nn_LNKillingRelu: out = where(kf<=0, x, x + kf*d)
#   d  = einsum('fkn,gf->gkn', x, W)                      (per batch)
#   kf = einsum('fkn,kl,fln->fn', x, G, d)  broadcast over k
# G is the (constant) Killing-form Gram matrix of sl(3):
#   G[0,0]=G[4,4]=12, G[0,4]=G[4,0]=-6, G[1,3]=G[3,1]=G[2,6]=G[6,2]=G[5,7]=G[7,5]=6
# so with kf' = kf/6 and z = (G/6)x along k:
#   kf' = z0*d0 + x3*d1 + x6*d2 + x1*d3 + z4*d4 + x7*d5 + x2*d6 + x5*d7
#   out = x + relu(6*kf') * d
# where z0 = 2x0-x4, z4 = 2x4-x0 are shipped as two extra host-computed
# planes (x dram layout [F, 10, N]: planes 0-7 = x, 8 = z0, 9 = z4).
#
# fp16 everywhere off-PSUM: matmul runs at 4x the fp32 PE rate, DVE gets its
# 2x packed mode, DMA halves.  d is accumulated fp32 in PSUM and copied to
# SBUF fp16 by the scalar engine.  Verified numerics: rel err ~1.2e-3.
#
# All elementwise work is DVE+ACT only: GpSimd shares its SBUF port pair with
# the DVE perf modes (exclusive lock per instruction), so any GpSimd activity
# stalls packed DVE ops 3-6x -- measured net negative.
#
# Tiles are double-width (two 256-column PSUM matmul blocks feed one 512-wide
# elementwise tile) to halve the per-instruction DVE overhead.
#
# Sharding: data-parallel over batch B=8 -> one batch per NeuronCore (8 cores).
# W is replicated (host passes W^T in fp16 so lhsT chunks slice directly).

from contextlib import ExitStack

import numpy as np

import concourse.bass as bass
import concourse.mybir as mybir
import concourse.tile as tile
from concourse.bass_utils import run_bass_kernel_spmd

B, F, K, N = 8, 512, 8, 2048
KP = 10  # 8 x-planes + 2 host-computed z-planes
P = 128
FT = F // P  # 4 channel tiles

f32 = mybir.dt.float32
f16 = mybir.dt.float16
Alu = mybir.AluOpType
ActF = mybir.ActivationFunctionType


def _ap(base, off_elems, dims):
    """Raw AP from a base AP: keep partition dim, replace free dims."""
    return bass.AP(
        tensor=base.tensor,
        offset=base.offset + off_elems,
        ap=[base.ap[0]] + dims,
    )


def build_nc(n_total=N, nt=256, pair=2):
    w2 = nt * pair  # elementwise tile width (pair matmul blocks wide)
    npc = n_total // w2  # pair-chunks
    nc = bass.Bass(detect_race_conditions=False)
    x = nc.dram_tensor("x", [F, KP, n_total], f16, kind="ExternalInput")
    wt = nc.dram_tensor("wt", [F, F], f16, kind="ExternalInput")  # W^T (f, g)
    out = nc.dram_tensor("out", [F, K, n_total], f16, kind="ExternalOutput")

    with tile.TileContext(nc) as tc, ExitStack() as ctx:
        wpool = ctx.enter_context(tc.tile_pool(name="w", bufs=1))
        xpool = ctx.enter_context(tc.tile_pool(name="xc", bufs=2))
        ppool = ctx.enter_context(tc.tile_pool(name="pd", bufs=2, space="PSUM"))
        dpool = ctx.enter_context(tc.tile_pool(name="dsb", bufs=3))
        prpool = ctx.enter_context(tc.tile_pool(name="prod", bufs=2))
        spool = ctx.enter_context(tc.tile_pool(name="small", bufs=2))
        gpool = ctx.enter_context(tc.tile_pool(name="gate", bufs=3))
        ogpool = ctx.enter_context(tc.tile_pool(name="og", bufs=2))
        opool = ctx.enter_context(tc.tile_pool(name="outs", bufs=3))

        # resident W^T tiles: wsb[ft][p, g] , f = ft*128+p
        # W goes FIRST, split across both DMA queues: it is tiny (0.5 MB) but
        # gates the first matmul block, so it must not queue behind chunk-0.
        wsb = []
        for ft in range(FT):
            w_t = wpool.tile([P, F], f16, tag=f"w{ft}")
            eng = nc.sync if ft < 2 else nc.scalar
            eng.dma_start(out=w_t[:], in_=wt[ft * P : (ft + 1) * P, :])
            wsb.append(w_t)

        # Dependency-free PE warm-up: ~4.5us of throwaway matmuls on a
        # memset tile keep the PE continuously busy while W / chunk-0 DMAs
        # land, so HAM clocks up to 2.4GHz before the first real block.
        warm = ppool.tile([P, K, nt], f32, tag="pd")
        dummy = wpool.tile([P, 512], f16, tag="dummy")
        nc.vector.memset(dummy[:], 0)
        for _ in range(10):
            nc.tensor.matmul(
                warm[:, 0:2, :], dummy[:, 0:P], dummy[:], start=True, stop=True
            )


        nmm = (K * nt) // 512  # 512-elem free chunks (one PSUM bank each)
        kper = 512 // nt  # k planes per matmul chunk

        def load_chunk(pc, dual_queue=False):
            xcs = []
            for ft in range(FT):
                xt = xpool.tile([P, KP, w2], f16, tag=f"xc{ft}")
                eng = nc.scalar if (dual_queue and ft % 2) else nc.sync
                eng.dma_start(
                    out=xt[:],
                    in_=x[ft * P : (ft + 1) * P, :, pc * w2 : (pc + 1) * w2],
                )
                xcs.append(xt)
            return xcs

        xchunks = {}
        ntile = npc * FT
        st = {}

        for it in range(ntile + 2):
            # ---- stage A: matmuls (pair halves) + d-copies for tile t = it ----
            if it < ntile:
                pc, gt = divmod(it, FT)
                if gt == 0 and pc == 0:
                    xchunks[0] = load_chunk(0, dual_queue=True)
                xcs = xchunks[pc]

                # d_sb free layout [K, w2]; half h fills columns [h*nt,(h+1)*nt)
                dsb = dpool.tile([P, K, w2], f16, tag="dsb")
                for h in range(pair):
                    pd = ppool.tile([P, K, nt], f32, tag="pd")
                    # Dummy first matmul absorbs the PSUM-slot-release wait so
                    # the first real matmul only waits on its x DMA.
                    nc.tensor.matmul(
                        pd[:, 0, 0:1], wsb[0][:, 0:P], wsb[0][:, 0:1],
                        start=True, stop=True,
                    )
                    # ft outer: same lhsT for nmm consecutive matmuls
                    for ft in range(FT):
                        if it == 0 and h == 0:
                            # Walrus only allows ONE sync wait per Matmult
                            # (waits ride the LDWEIGHTS struct).  A tiny warm
                            # matmul right before each ft group makes PE
                            # observe that W-DMA semaphore here, without
                            # stalling earlier groups on later W tiles.
                            nc.tensor.matmul(
                                warm[:, 0, 0:1], wsb[ft][:, 0:P], wsb[ft][:, 0:1],
                                start=True, stop=True, skip_group_check=True,
                            )
                        for jj in range(nmm):
                            nc.tensor.matmul(
                                pd[:, jj * kper : (jj + 1) * kper, :],
                                wsb[ft][:, gt * P : (gt + 1) * P],
                                _ap(
                                    xcs[ft][:],
                                    jj * kper * w2 + h * nt,
                                    [[w2, kper], [1, nt]],
                                ),
                                start=(ft == 0),
                                stop=(ft == FT - 1),
                            )
                    # PSUM(fp32) -> SBUF fp16 on the scalar engine
                    nc.scalar.activation(
                        out=_ap(dsb[:], h * nt, [[w2, K], [1, nt]]),
                        in_=pd[:],
                        func=ActF.Copy,
                    )
                st[it] = dict(pc=pc, gt=gt, dsb=dsb)

                if gt == 0 and pc + 1 < npc:
                    xchunks[pc + 1] = load_chunk(pc + 1)

            # ---- stage C: gate-mul + add-x + store for tile u = it - 2 ----
            u = it - 2
            if u >= 0:
                s = st.pop(u)
                pc, gt, dsb, gate = s["pc"], s["gt"], s["dsb"], s["gate"]
                xg = xchunks[pc][gt][:]
                dsb_ = dsb[:]

                # Last tile is processed per half so the final store overlaps
                # the remaining DVE work (shorter drain tail).
                og = ogpool.tile([P, K, w2], f16, tag="og")
                ot = opool.tile([P, K, w2], f16, tag="ot")
                for off, wd in ([(0, nt), (nt, nt)] if u == ntile - 1 else [(0, w2)]):
                    nc.vector.tensor_tensor(
                        out=_ap(og[:], off, [[w2, K], [1, wd]]),
                        in0=_ap(gate[:], off, [[0, K], [1, wd]]),
                        in1=_ap(dsb_, off, [[w2, K], [1, wd]]),
                        op=Alu.mult,
                    )
                    nc.vector.tensor_tensor(
                        out=_ap(ot[:], off, [[w2, K], [1, wd]]),
                        in0=_ap(og[:], off, [[w2, K], [1, wd]]),
                        in1=_ap(xg, off, [[w2, K], [1, wd]]),
                        op=Alu.add,
                    )
                    nc.scalar.dma_start(
                        out=out[
                            gt * P : (gt + 1) * P,
                            :,
                            pc * w2 + off : pc * w2 + off + wd,
                        ],
                        in_=_ap(ot[:], off, [[w2, K], [1, wd]]),
                    )

            # ---- stage B: products + tree + gate for tile v = it - 1 ----
            v = it - 1
            if 0 <= v < ntile:
                s = st[v]
                pc, gt, dsb = s["pc"], s["gt"], s["dsb"]
                xg = xchunks[pc][gt][:]
                dsb_ = dsb[:]

                # products p_k = z_k * d_k; z is a plane-permutation of x
                # except planes 0,4 which are the host-shipped planes 8,9.
                # For the pipeline-priming tile the ops are split per matmul
                # half so the DVE starts right after the first d-copy.
                p = prpool.tile([P, K, w2], f16, tag="p")
                for off, wd in ([(0, nt), (nt, nt)] if v == 0 else [(0, w2)]):
                    # k in (1,3,5,7): z_k = x at (3,1,7,5)
                    nc.vector.tensor_tensor(
                        out=_ap(p[:], w2 + off, [[4 * w2, 2], [2 * w2, 2], [1, wd]]),
                        in0=_ap(xg, 3 * w2 + off, [[4 * w2, 2], [-2 * w2, 2], [1, wd]]),
                        in1=_ap(dsb_, w2 + off, [[4 * w2, 2], [2 * w2, 2], [1, wd]]),
                        op=Alu.mult,
                    )
                    # k in (2,6): z_k = x at (6,2)
                    nc.vector.tensor_tensor(
                        out=_ap(p[:], 2 * w2 + off, [[4 * w2, 2], [1, wd]]),
                        in0=_ap(xg, 6 * w2 + off, [[-4 * w2, 2], [1, wd]]),
                        in1=_ap(dsb_, 2 * w2 + off, [[4 * w2, 2], [1, wd]]),
                        op=Alu.mult,
                    )
                    # k in (0,4): z planes 8,9
                    nc.vector.tensor_tensor(
                        out=_ap(p[:], off, [[4 * w2, 2], [1, wd]]),
                        in0=_ap(xg, 8 * w2 + off, [[w2, 2], [1, wd]]),
                        in1=_ap(dsb_, off, [[4 * w2, 2], [1, wd]]),
                        op=Alu.mult,
                    )

                # ---- kf' = sum_k p_k as a pairwise tree (keeps step-1 reads
                #      so the DVE 2x packed mode stays engaged) ----
                s1 = spool.tile([P, 4, w2], f16, tag="s1")
                nc.vector.tensor_tensor(
                    out=s1[:], in0=p[:, 0:4, :], in1=p[:, 4:8, :], op=Alu.add
                )
                s2 = spool.tile([P, 2, w2], f16, tag="s2")
                nc.vector.tensor_tensor(
                    out=s2[:], in0=s1[:, 0:2, :], in1=s1[:, 2:4, :], op=Alu.add
                )
                kf = spool.tile([P, w2], f16, tag="kf")
                nc.vector.tensor_tensor(
                    out=kf[:], in0=s2[:, 0, :], in1=s2[:, 1, :], op=Alu.add
                )

                # ---- gate = relu(6 * kf') on ScalarE ----
                gate = gpool.tile([P, w2], f16, tag="gate")
                nc.scalar.activation(out=gate[:], in_=kf[:], func=ActF.Relu, scale=6.0)
                s["gate"] = gate

    _split_waits(nc)
    return nc


# Engine datapath structs (Matmult/TT/STT/Act/...) only carry ONE sync wait on
# TRN2 walrus; sequencer instructions (NoOp) can each carry one more.  Hoist
# surplus waits onto same-engine NoOps placed just before the instruction.
_SEQ_OK = set()  # every struct on this walrus takes at most ONE sync wait


def _split_waits(nc):
    nnop = 0
    for fn in nc.m.functions:
        for blk in fn.blocks:
            out = []
            for inst in blk.instructions:
                si = inst.sync_info
                if (
                    si is not None
                    and si.on_wait
                    and len(si.on_wait) > 1
                    and type(inst).__name__ not in _SEQ_OK
                ):
                    for w in si.on_wait[:-1]:
                        nop = mybir.InstNoOp(
                            name=f"{inst.name}-sw{nnop}",
                            opcode="NoOp",
                            engine=inst.engine,
                            sync_info=mybir.SyncInfo(on_wait=[w], on_update=[]),
                        )
                        nnop += 1
                        out.append(nop)
                    inst.sync_info = mybir.SyncInfo(
                        on_wait=[si.on_wait[-1]], on_update=list(si.on_update)
                    )
                out.append(inst)
            blk.instructions[:] = out
    return nc


_NC_CACHE = {}


def _get_nc(n_total=N, nt=256):
    key = (n_total, nt)
    if key not in _NC_CACHE:
        _NC_CACHE[key] = build_nc(n_total, nt)
    return _NC_CACHE[key]


def prep_inputs(x: np.ndarray, W: np.ndarray):
    """Host-side prep: fp16 conversion, W transpose, and the two z-planes."""
    wt16 = np.ascontiguousarray(W.T.astype(np.float16))
    xa = np.empty((B, F, KP, N), np.float16)
    xa[:, :, 0:8] = x.astype(np.float16)
    xa[:, :, 8] = (2.0 * x[:, :, 0] - x[:, :, 4]).astype(np.float16)
    xa[:, :, 9] = (2.0 * x[:, :, 4] - x[:, :, 0]).astype(np.float16)
    return [
        {"x": np.ascontiguousarray(xa[b]), "wt": wt16} for b in range(B)
    ]


def kernel(x: np.ndarray, W: np.ndarray) -> np.ndarray:
    assert x.shape == (B, F, K, N) and W.shape == (F, F)
    in_maps = prep_inputs(x, W)
    nc = _get_nc()
    res = run_bass_kernel_spmd(nc, in_maps, list(range(B)))
    return np.stack(
        [res.results[b]["out"] for b in range(B)], axis=0
    ).astype(np.float32)


if __name__ == "__main__":
    xs = np.random.randn(B, F, K, N).astype(np.float32)
    Ws = (np.random.randn(F, F) / np.sqrt(F)).astype(np.float32)
    o = kernel(xs, Ws)
    print(o.shape, o.dtype)


# revision 19
# speedup vs baseline: 1.1708x; 1.1708x over previous
# Trainium2 Bass kernel for nn_LNKillingRelu: out = where(kf<=0, x, x + kf*d)
#   d  = einsum('fkn,gf->gkn', x, W)                      (per batch)
#   kf = einsum('fkn,kl,fln->fn', x, G, d)  broadcast over k
# G is the (constant) Killing-form Gram matrix of sl(3):
#   G[0,0]=G[4,4]=12, G[0,4]=G[4,0]=-6, G[1,3]=G[3,1]=G[2,6]=G[6,2]=G[5,7]=G[7,5]=6
# so with kf' = kf/6 and z = (G/6)x along k:
#   kf' = z0*d0 + x3*d1 + x6*d2 + x1*d3 + z4*d4 + x7*d5 + x2*d6 + x5*d7
#   out = x + relu(6*kf') * d
# where z0 = 2x0-x4, z4 = 2x4-x0 are shipped as two extra host-computed
# planes (x dram layout [F, 10, N]: planes 0-7 = x, 8 = z0, 9 = z4).
#
# fp16 everywhere off-PSUM: matmul runs at 4x the fp32 PE rate, DVE gets its
# 2x packed mode, DMA halves.  d is accumulated fp32 in PSUM and copied to
# SBUF fp16 by the scalar engine.  Verified numerics: rel err ~1.2e-3.
#
# All elementwise work is DVE+ACT only: GpSimd shares its SBUF port pair with
# the DVE perf modes (exclusive lock per instruction), so any GpSimd activity
# stalls packed DVE ops 3-6x -- measured net negative.
#
# Tiles are double-width (two 256-column PSUM matmul blocks feed one 512-wide
# elementwise tile) to halve the per-instruction DVE overhead.
#
# Sharding: data-parallel over batch B=8 -> one batch per NeuronCore (8 cores).
# W is replicated (host passes W^T in fp16 so lhsT chunks slice directly).

from contextlib import ExitStack

import numpy as np

import concourse.bass as bass
import concourse.mybir as mybir
import concourse.tile as tile
from concourse.bass_utils import run_bass_kernel_spmd

B, F, K, N = 8, 512, 8, 2048
KP = 10  # 8 x-planes + 2 host-computed z-planes
P = 128
FT = F // P  # 4 channel tiles

f32 = mybir.dt.float32
f16 = mybir.dt.float16
Alu = mybir.AluOpType
ActF = mybir.ActivationFunctionType


def _ap(base, off_elems, dims):
    """Raw AP from a base AP: keep partition dim, replace free dims."""
    return bass.AP(
        tensor=base.tensor,
        offset=base.offset + off_elems,
        ap=[base.ap[0]] + dims,
    )


def build_nc(n_total=N, nt=256, pair=2):
    w2 = nt * pair  # elementwise tile width (pair matmul blocks wide)
    npc = n_total // w2  # pair-chunks
    nc = bass.Bass(detect_race_conditions=False)
    x = nc.dram_tensor("x", [F, KP, n_total], f16, kind="ExternalInput")
    wt = nc.dram_tensor("wt", [F, F], f16, kind="ExternalInput")  # W^T (f, g)
    out = nc.dram_tensor("out", [F, K, n_total], f16, kind="ExternalOutput")

    with tile.TileContext(nc) as tc, ExitStack() as ctx:
        wpool = ctx.enter_context(tc.tile_pool(name="w", bufs=1))
        xpool = ctx.enter_context(tc.tile_pool(name="xc", bufs=2))
        ppool = ctx.enter_context(tc.tile_pool(name="pd", bufs=2, space="PSUM"))
        dpool = ctx.enter_context(tc.tile_pool(name="dsb", bufs=3))
        prpool = ctx.enter_context(tc.tile_pool(name="prod", bufs=2))
        spool = ctx.enter_context(tc.tile_pool(name="small", bufs=2))
        gpool = ctx.enter_context(tc.tile_pool(name="gate", bufs=3))
        ogpool = ctx.enter_context(tc.tile_pool(name="og", bufs=2))
        opool = ctx.enter_context(tc.tile_pool(name="outs", bufs=3))

        # resident W^T tiles: wsb[ft][p, g] , f = ft*128+p
        # W goes FIRST, split across both DMA queues: it is tiny (0.5 MB) but
        # gates the first matmul block, so it must not queue behind chunk-0.
        wsb = []
        for ft in range(FT):
            w_t = wpool.tile([P, F], f16, tag=f"w{ft}")
            eng = nc.sync if ft < 2 else nc.scalar
            eng.dma_start(out=w_t[:], in_=wt[ft * P : (ft + 1) * P, :])
            wsb.append(w_t)

        # Dependency-free PE warm-up: ~4.5us of throwaway matmuls on a
        # memset tile keep the PE continuously busy while W / chunk-0 DMAs
        # land, so HAM clocks up to 2.4GHz before the first real block.
        warm = ppool.tile([P, K, nt], f32, tag="pd")
        dummy = wpool.tile([P, 512], f16, tag="dummy")
        nc.vector.memset(dummy[:], 0)
        for _ in range(10):
            nc.tensor.matmul(
                warm[:, 0:2, :], dummy[:, 0:P], dummy[:], start=True, stop=True
            )




        nmm = (K * nt) // 512  # 512-elem free chunks (one PSUM bank each)
        kper = 512 // nt  # k planes per matmul chunk

        def load_chunk(pc, dual_queue=False):
            xcs = []
            for ft in range(FT):
                xt = xpool.tile([P, KP, w2], f16, tag=f"xc{ft}")
                eng = nc.scalar if (dual_queue and ft % 2) else nc.sync
                eng.dma_start(
                    out=xt[:],
                    in_=x[ft * P : (ft + 1) * P, :, pc * w2 : (pc + 1) * w2],
                )
                xcs.append(xt)
            return xcs

        xchunks = {}
        ntile = npc * FT
        st = {}

        for it in range(ntile + 2):
            # ---- stage A: matmuls (pair halves) + d-copies for tile t = it ----
            if it < ntile:
                pc, gt = divmod(it, FT)
                if gt == 0 and pc == 0:
                    xchunks[0] = load_chunk(0, dual_queue=True)
                xcs = xchunks[pc]

                # d_sb free layout [K, w2]; half h fills columns [h*nt,(h+1)*nt)
                dsb = dpool.tile([P, K, w2], f16, tag="dsb")
                for h in range(pair):
                    pd = ppool.tile([P, K, nt], f32, tag="pd")
                    # Dummy first matmul absorbs the PSUM-slot-release wait so
                    # the first real matmul only waits on its x DMA.
                    nc.tensor.matmul(
                        pd[:, 0, 0:1], wsb[0][:, 0:P], wsb[0][:, 0:1],
                        start=True, stop=True,
                    )
                    # ft outer: same lhsT for nmm consecutive matmuls
                    for ft in range(FT):
                        if it == 0 and h == 0:
                            # Walrus only allows ONE sync wait per Matmult
                            # (waits ride the LDWEIGHTS struct).  A standalone
                            # ldweights right before each ft group makes PE
                            # observe that W-DMA semaphore here -- without
                            # stalling earlier groups on later W tiles, and
                            # with no PSUM side effects.
                            nc.tensor.ldweights(wsb[ft][:, gt * P : (gt + 1) * P])
                        for jj in range(nmm):
                            nc.tensor.matmul(
                                pd[:, jj * kper : (jj + 1) * kper, :],
                                wsb[ft][:, gt * P : (gt + 1) * P],
                                _ap(
                                    xcs[ft][:],
                                    jj * kper * w2 + h * nt,
                                    [[w2, kper], [1, nt]],
                                ),
                                start=(ft == 0),
                                stop=(ft == FT - 1),
                            )
                    # PSUM(fp32) -> SBUF fp16 on the scalar engine
                    nc.scalar.activation(
                        out=_ap(dsb[:], h * nt, [[w2, K], [1, nt]]),
                        in_=pd[:],
                        func=ActF.Copy,
                    )
                st[it] = dict(pc=pc, gt=gt, dsb=dsb)

                if gt == 0 and pc + 1 < npc:
                    xchunks[pc + 1] = load_chunk(pc + 1)

            # ---- stage C: gate-mul + add-x + store for tile u = it - 2 ----
            u = it - 2
            if u >= 0:
                s = st.pop(u)
                pc, gt, dsb, gate = s["pc"], s["gt"], s["dsb"], s["gate"]
                xg = xchunks[pc][gt][:]
                dsb_ = dsb[:]

                # Last tile is processed per half so the final store overlaps
                # the remaining DVE work (shorter drain tail).
                og = ogpool.tile([P, K, w2], f16, tag="og")
                ot = opool.tile([P, K, w2], f16, tag="ot")
                for off, wd in ([(0, nt), (nt, nt)] if u == ntile - 1 else [(0, w2)]):
                    nc.vector.tensor_tensor(
                        out=_ap(og[:], off, [[w2, K], [1, wd]]),
                        in0=_ap(gate[:], off, [[0, K], [1, wd]]),
                        in1=_ap(dsb_, off, [[w2, K], [1, wd]]),
                        op=Alu.mult,
                    )
                    nc.vector.tensor_tensor(
                        out=_ap(ot[:], off, [[w2, K], [1, wd]]),
                        in0=_ap(og[:], off, [[w2, K], [1, wd]]),
                        in1=_ap(xg, off, [[w2, K], [1, wd]]),
                        op=Alu.add,
                    )
                    nc.scalar.dma_start(
                        out=out[
                            gt * P : (gt + 1) * P,
                            :,
                            pc * w2 + off : pc * w2 + off + wd,
                        ],
                        in_=_ap(ot[:], off, [[w2, K], [1, wd]]),
                    )

            # ---- stage B: products + tree + gate for tile v = it - 1 ----
            v = it - 1
            if 0 <= v < ntile:
                s = st[v]
                pc, gt, dsb = s["pc"], s["gt"], s["dsb"]
                xg = xchunks[pc][gt][:]
                dsb_ = dsb[:]

                # products p_k = z_k * d_k; z is a plane-permutation of x
                # except planes 0,4 which are the host-shipped planes 8,9.
                # For the pipeline-priming tile the ops are split per matmul
                # half so the DVE starts right after the first d-copy.
                p = prpool.tile([P, K, w2], f16, tag="p")
                for off, wd in ([(0, nt), (nt, nt)] if v == 0 else [(0, w2)]):
                    # k in (1,3,5,7): z_k = x at (3,1,7,5)
                    nc.vector.tensor_tensor(
                        out=_ap(p[:], w2 + off, [[4 * w2, 2], [2 * w2, 2], [1, wd]]),
                        in0=_ap(xg, 3 * w2 + off, [[4 * w2, 2], [-2 * w2, 2], [1, wd]]),
                        in1=_ap(dsb_, w2 + off, [[4 * w2, 2], [2 * w2, 2], [1, wd]]),
                        op=Alu.mult,
                    )
                    # k in (2,6): z_k = x at (6,2)
                    nc.vector.tensor_tensor(
                        out=_ap(p[:], 2 * w2 + off, [[4 * w2, 2], [1, wd]]),
                        in0=_ap(xg, 6 * w2 + off, [[-4 * w2, 2], [1, wd]]),
                        in1=_ap(dsb_, 2 * w2 + off, [[4 * w2, 2], [1, wd]]),
                        op=Alu.mult,
                    )
                    # k in (0,4): z planes 8,9
                    nc.vector.tensor_tensor(
                        out=_ap(p[:], off, [[4 * w2, 2], [1, wd]]),
                        in0=_ap(xg, 8 * w2 + off, [[w2, 2], [1, wd]]),
                        in1=_ap(dsb_, off, [[4 * w2, 2], [1, wd]]),
                        op=Alu.mult,
                    )

                # ---- kf' = sum_k p_k as a pairwise tree (keeps step-1 reads
                #      so the DVE 2x packed mode stays engaged) ----
                s1 = spool.tile([P, 4, w2], f16, tag="s1")
                nc.vector.tensor_tensor(
                    out=s1[:], in0=p[:, 0:4, :], in1=p[:, 4:8, :], op=Alu.add
                )
                s2 = spool.tile([P, 2, w2], f16, tag="s2")
                nc.vector.tensor_tensor(
                    out=s2[:], in0=s1[:, 0:2, :], in1=s1[:, 2:4, :], op=Alu.add
                )
                kf = spool.tile([P, w2], f16, tag="kf")
                nc.vector.tensor_tensor(
                    out=kf[:], in0=s2[:, 0, :], in1=s2[:, 1, :], op=Alu.add
                )

                # ---- gate = relu(6 * kf') on ScalarE ----
                gate = gpool.tile([P, w2], f16, tag="gate")
                nc.scalar.activation(out=gate[:], in_=kf[:], func=ActF.Relu, scale=6.0)
                s["gate"] = gate

    _split_waits(nc)
    return nc


# Engine datapath structs (Matmult/TT/STT/Act/...) only carry ONE sync wait on
# TRN2 walrus; sequencer instructions (NoOp) can each carry one more.  Hoist
# surplus waits onto same-engine NoOps placed just before the instruction.
_SEQ_OK = set()  # every struct on this walrus takes at most ONE sync wait


def _split_waits(nc):
    nnop = 0
    for fn in nc.m.functions:
        for blk in fn.blocks:
            out = []
            for inst in blk.instructions:
                si = inst.sync_info
                if (
                    si is not None
                    and si.on_wait
                    and len(si.on_wait) > 1
                    and type(inst).__name__ not in _SEQ_OK
                ):
                    for w in si.on_wait[:-1]:
                        nop = mybir.InstNoOp(
                            name=f"{inst.name}-sw{nnop}",
                            opcode="NoOp",
                            engine=inst.engine,
                            sync_info=mybir.SyncInfo(on_wait=[w], on_update=[]),
                        )
                        nnop += 1
                        out.append(nop)
                    inst.sync_info = mybir.SyncInfo(
                        on_wait=[si.on_wait[-1]], on_update=list(si.on_update)
                    )
                out.append(inst)
            blk.instructions[:] = out
    return nc


_NC_CACHE = {}


def _get_nc(n_total=N, nt=256):
    key = (n_total, nt)
    if key not in _NC_CACHE:
        _NC_CACHE[key] = build_nc(n_total, nt)
    return _NC_CACHE[key]


def prep_inputs(x: np.ndarray, W: np.ndarray):
    """Host-side prep: fp16 conversion, W transpose, and the two z-planes."""
    wt16 = np.ascontiguousarray(W.T.astype(np.float16))
    xa = np.empty((B, F, KP, N), np.float16)
    xa[:, :, 0:8] = x.astype(np.float16)
    xa[:, :, 8] = (2.0 * x[:, :, 0] - x[:, :, 4]).astype(np.float16)
    xa[:, :, 9] = (2.0 * x[:, :, 4] - x[:, :, 0]).astype(np.float16)
    return [
        {"x": np.ascontiguousarray(xa[b]), "wt": wt16} for b in range(B)
    ]


def kernel(x: np.ndarray, W: np.ndarray) -> np.ndarray:
    assert x.shape == (B, F, K, N) and W.shape == (F, F)
    in_maps = prep_inputs(x, W)
    nc = _get_nc()
    res = run_bass_kernel_spmd(nc, in_maps, list(range(B)))
    return np.stack(
        [res.results[b]["out"] for b in range(B)], axis=0
    ).astype(np.float32)


if __name__ == "__main__":
    xs = np.random.randn(B, F, K, N).astype(np.float32)
    Ws = (np.random.randn(F, F) / np.sqrt(F)).astype(np.float32)
    o = kernel(xs, Ws)
    print(o.shape, o.dtype)
